# revision 24
# baseline (speedup 1.0000x reference)
"""MoE layer (8 experts, top-2) as an expert-parallel Trainium2 Bass kernel.

Strategy:
  - Host: gating matmul (tiny), top-2 routing, gather tokens per expert.
  - Device (8 NeuronCores, SPMD, one expert per core): fused FFN over the
    expert's tokens in 256-token blocks. Per block, for each 128-wide f-tile:
    mm1 (8 accumulating matmuls) -> relu+bias on ScalarE into a transient
    bf16 h-tile in SBUF -> mm2 immediately accumulates into 4 persistent
    PSUM banks. No h round-trip to DRAM (the whole FFN intermediate stays
    on-chip). W1, W2 and x are fully SBUF-resident.
  - Host: scatter-add the two expert contributions per token, add b2 term.

Layouts (device side, per core):
  xT : [128, 8*C]     bf16  xT[p, ko*C + c]            = x_tok[c, ko*128+p]
  w1 : [128, 32*8*128]bf16  w1[p, (ft*8+ko)*128 + f']  = W1[ko*128+p, ft*128+f']
  w2 : [128, 32*1024] bf16  w2[p, ft*1024 + d]         = W2[ft*128+p, d]
  b1 : [128, 32]      f32   b1[p, ft]                  = b1_vec[ft*128+p]
  cw : [128, C/128]   f32   cw[p, o]                   = combine_weight[o*128+p]
  y  : [128, (C/128)*1024] f32 (out)  y[p, o*D+d]      = y_tok[o*128+p, d]
"""

import os

import numpy as np
import ml_dtypes

D_MODEL = 1024
D_FF = 4096
N_EXPERTS = 8
TOP_K = 2
B, S = 4, 2048
T = B * S
P = 128
KO1 = D_MODEL // P   # 8  k-subtiles for matmul1
NFT = D_FF // P      # 32 f-tiles
N_CORES = 8
TB = 256             # token block

BF16 = ml_dtypes.bfloat16

# Compiled-module cache keyed by padded capacity C.
_NC_CACHE = {}
LAST_RESULTS = None  # BassKernelResults of the most recent run (for test.py)
LAST_IN_MAPS = None  # per-core input maps of the most recent run
LAST_C = None


def _groups_for(C):
    """Split C tokens (multiple of 128) into token groups.

    A small lead group shortens the ramp (less x to DMA before the first
    matmul); small trailing groups shorten the mm2-only epilogue."""
    assert C % P == 0
    sizes = []
    rem = C
    while rem >= 512:
        sizes.append(512)
        rem -= 512
    if rem:
        sizes.append(rem)
    groups = []
    off = 0
    for tb in sizes:
        groups.append((off, tb))
        off += tb
    return groups


def _pairs_for(mt):
    """Split a group's m-tiles into two halves (pairA runs in phase A)."""
    if mt >= 2:
        return list(range(mt // 2)), list(range(mt // 2, mt))
    return [0], []


def _build_nc(C, reps=1):
    import concourse.bass as bass  # noqa: F401
    import concourse.tile as tile
    from concourse import bacc, mybir
    from contextlib import ExitStack

    groups = _groups_for(C)
    OUTERS = C // P

    nc = bacc.Bacc("TRN2", target_bir_lowering=False, debug=False,
                   num_devices=N_CORES)

    xT = nc.dram_tensor("xT", [P, KO1 * C], mybir.dt.bfloat16,
                        kind="ExternalInput")
    w1 = nc.dram_tensor("w1", [P, NFT * KO1 * P], mybir.dt.bfloat16,
                        kind="ExternalInput")
    w2 = nc.dram_tensor("w2", [P, NFT * D_MODEL], mybir.dt.bfloat16,
                        kind="ExternalInput")
    b1 = nc.dram_tensor("b1", [P, NFT], mybir.dt.float32,
                        kind="ExternalInput")
    cw = nc.dram_tensor("cw", [P, OUTERS], mybir.dt.float32,
                        kind="ExternalInput")
    y = nc.dram_tensor("y", [P, OUTERS * D_MODEL], mybir.dt.float32,
                       kind="ExternalOutput")

    xT_ap = xT.ap().rearrange("p (ko c) -> p ko c", ko=KO1)
    w1_ap = w1.ap().rearrange("p (ft ko f) -> p ft ko f", ft=NFT, ko=KO1)
    w2_ap = w2.ap().rearrange("p (ft d) -> p ft d", ft=NFT)
    y_ap = y.ap()

    NG = len(groups)

    with tile.TileContext(nc) as tc, ExitStack() as ctx:
        wpool = ctx.enter_context(tc.tile_pool(name="wpool", bufs=1))
        xpool = ctx.enter_context(tc.tile_pool(name="xpool", bufs=2))
        hpool = ctx.enter_context(tc.tile_pool(name="hpool", bufs=1))
        ypool = ctx.enter_context(tc.tile_pool(name="ypool", bufs=3))
        ps1 = ctx.enter_context(tc.tile_pool(name="ps1", bufs=2, space="PSUM"))
        ps2 = ctx.enter_context(tc.tile_pool(name="ps2", bufs=6, space="PSUM"))

        for rep in range(reps):
            b1s = wpool.tile([P, NFT], mybir.dt.float32, tag="b1s",
                             name="b1s")
            cws = wpool.tile([P, OUTERS], mybir.dt.float32, tag="cws",
                             name="cws")

            # x group tiles rotate through 2 slots; group 0 first so mm1 can
            # start immediately.  DMA descriptor issue costs ~0.6us per
            # dma_start on a sequencer, so transfers are merged and the
            # ramp-critical ones are spread across two sequencers.
            def load_x(gi):
                c0, gtb = groups[gi]
                xt = xpool.tile([P, KO1, 512], mybir.dt.bfloat16, tag="xg",
                                name=f"x_{gi}")
                if gi == 0:
                    half = KO1 // 2
                    nc.sync.dma_start(xt[:, :half, :gtb],
                                      xT_ap[:, :half, c0:c0 + gtb])
                    nc.gpsimd.dma_start(xt[:, half:, :gtb],
                                        xT_ap[:, half:, c0:c0 + gtb])
                else:
                    nc.sync.dma_start(xt[:, :, :gtb],
                                      xT_ap[:, :, c0:c0 + gtb])
                return xt

            # Weights in ft order, W1 first (group 0 only runs mm1; W2 is
            # not needed until its mm2 a full group later).  w1[0] + x(0)
            # are the ramp critical path.
            w1t, w2t = [], []
            a = wpool.tile([P, KO1, P], mybir.dt.bfloat16, tag="w1_0",
                           name="w1_0")
            nc.sync.dma_start(a[:], w1_ap[:, 0])
            w1t.append(a)
            x_pending = {0: load_x(0)}
            nc.gpsimd.dma_start(b1s[:], b1.ap())
            for ft in range(1, NFT):
                a = wpool.tile([P, KO1, P], mybir.dt.bfloat16,
                               tag=f"w1_{ft}", name=f"w1_{ft}")
                eng = nc.gpsimd if ft % 2 else nc.sync
                eng.dma_start(a[:], w1_ap[:, ft])
                w1t.append(a)
            for ft in range(NFT):
                b = wpool.tile([P, D_MODEL], mybir.dt.bfloat16,
                               tag=f"w2_{ft}", name=f"w2_{ft}")
                eng = nc.gpsimd if ft % 2 else nc.sync
                eng.dma_start(b[:], w2_ap[:, ft])
                w2t.append(b)
            nc.sync.dma_start(cws[:], cw.ap())

            for gi in range(1, NG):
                x_pending[gi] = load_x(gi)

            # h tiles: one resident slot per ft, rewritten each group.  The
            # phase structure (mm2 of the previous group reads h[ft] before
            # this group's relu rewrites it) makes a single buffer safe.
            def h_tile(ft):
                return hpool.tile([P, 512], mybir.dt.bfloat16,
                                  tag=f"h_{ft}", name=f"h_{ft}")

            def emit_mm2(pts2, h_prev, prev_mt, pair, ft):
                for m in pair:
                    for half in range(2):
                        nc.tensor.matmul(
                            pts2[2 * m + half],
                            h_prev[ft][:, m * P:(m + 1) * P],
                            w2t[ft][:, half * 512:(half + 1) * 512],
                            start=(ft == 0),
                            stop=(ft == NFT - 1),
                        )

            def emit_yscale(pts2, c0, pair):
                for m in pair:
                    outer = c0 // P + m
                    yt = ypool.tile([P, D_MODEL], mybir.dt.float32, tag="yt",
                                    name="yt")
                    for half in range(2):
                        nc.vector.tensor_scalar_mul(
                            yt[:, half * 512:(half + 1) * 512],
                            pts2[2 * m + half],
                            cws[:, outer:outer + 1],
                        )
                    nc.sync.dma_start(
                        y_ap[:, outer * D_MODEL:(outer + 1) * D_MODEL],
                        yt[:],
                    )

            h_cur = None
            h_prev = None
            pts2_prev = None
            for gi in range(NG + 1):
                cur = groups[gi] if gi < NG else None
                prev = groups[gi - 1] if gi > 0 else None
                if prev is not None:
                    prev_mt = prev[1] // P
                    pairA, pairB = _pairs_for(prev_mt)
                    pts2_prev = [ps2.tile([P, 512], mybir.dt.float32,
                                          tag="ps2", name="pt2")
                                 for _ in range(2 * prev_mt)]
                if cur is None and prev is not None:
                    # Final group: bank-major mm2 so each bank's y
                    # writeback overlaps the remaining banks' matmuls,
                    # shrinking the post-matmul tail.
                    for m in pairA + pairB:
                        outer = prev[0] // P + m
                        for half in range(2):
                            for ft in range(NFT):
                                nc.tensor.matmul(
                                    pts2_prev[2 * m + half],
                                    h_prev[ft][:, m * P:(m + 1) * P],
                                    w2t[ft][:, half * 512:(half + 1) * 512],
                                    start=(ft == 0),
                                    stop=(ft == NFT - 1),
                                )
                            yt = ypool.tile([P, 512], mybir.dt.float32,
                                            tag="yt", name="yt")
                            nc.vector.tensor_scalar_mul(
                                yt[:], pts2_prev[2 * m + half],
                                cws[:, outer:outer + 1],
                            )
                            nc.sync.dma_start(
                                y_ap[:, outer * D_MODEL + half * 512:
                                     outer * D_MODEL + (half + 1) * 512],
                                yt[:],
                            )
                    continue
                # Phase A: mm2 of the previous group's first m-pair (pure
                # mm2; frees h[ft] early is NOT yet true -- pairB still
                # reads h in phase B).
                if prev is not None:
                    for ft in range(NFT):
                        emit_mm2(pts2_prev, h_prev, prev_mt, pairA, ft)
                    emit_yscale(pts2_prev, prev[0], pairA)
                # Phase B: mm2 of previous group's second m-pair (last h
                # reader), then this group's mm1 + relu into the freed h.
                if cur is not None:
                    c0, gtb = cur
                    mt = gtb // P
                    xt = x_pending[gi]
                    h_cur = []
                for ft in range(NFT):
                    if prev is not None and pairB:
                        emit_mm2(pts2_prev, h_prev, prev_mt, pairB, ft)
                    if cur is not None:
                        pt = ps1.tile([P, 512], mybir.dt.float32, tag="ps1",
                                      name="pt1")
                        for ko in range(KO1):
                            nc.tensor.matmul(
                                pt[:, :gtb],
                                w1t[ft][:, ko, :],
                                xt[:, ko, :gtb],
                                start=(ko == 0),
                                stop=(ko == KO1 - 1),
                            )
                        hf = h_tile(ft)
                        nc.scalar.activation(
                            hf[:, :gtb], pt[:, :gtb],
                            mybir.ActivationFunctionType.Relu,
                            bias=b1s[:, ft:ft + 1],
                        )
                        h_cur.append(hf)
                if prev is not None and pairB:
                    emit_yscale(pts2_prev, prev[0], pairB)
                if cur is not None:
                    h_prev = h_cur

    nc.compile()
    return nc


def _route(x_flat, Wg, bg):
    logits = x_flat.astype(np.float32) @ Wg.astype(np.float32) + bg
    idx = np.argsort(-logits, axis=1, kind="stable")[:, :TOP_K]
    gates = np.take_along_axis(logits, idx, axis=1)  # [T, 2] descending
    e1 = np.exp(gates[:, 1] - gates[:, 0])
    denom = 1.0 + e1
    w = np.stack([1.0 / denom, e1 / denom], axis=1).astype(np.float32)
    return idx.astype(np.int32), w


def kernel(x, Wg, bg, W1, b1, W2, b2):
    global LAST_RESULTS
    x = np.asarray(x, dtype=np.float32)
    Wg = np.asarray(Wg, dtype=np.float32)
    bg = np.asarray(bg, dtype=np.float32)
    W1 = np.asarray(W1, dtype=np.float32)
    b1 = np.asarray(b1, dtype=np.float32)
    W2 = np.asarray(W2, dtype=np.float32)
    b2 = np.asarray(b2, dtype=np.float32)

    x_flat = x.reshape(T, D_MODEL)
    idx, w = _route(x_flat, Wg, bg)

    # Per-expert token lists + slot map (position of each (token, k) pair
    # inside its expert's gathered block).
    tok_lists = []
    counts = []
    slot = np.empty((T, TOP_K), dtype=np.int64)
    for e in range(N_EXPERTS):
        mask = (idx[:, 0] == e) | (idx[:, 1] == e)
        tok = np.nonzero(mask)[0]
        tok_lists.append(tok)
        counts.append(len(tok))
        which = (idx[tok, 1] == e).astype(np.int64)  # 0 if k=0 slot, else 1
        slot[tok, which] = np.arange(len(tok))

    C = max(counts)
    C = ((C + P - 1) // P) * P

    if C not in _NC_CACHE:
        _NC_CACHE[C] = _build_nc(C)
    nc = _NC_CACHE[C]

    # Build per-core input maps.
    in_maps = []
    for e in range(N_EXPERTS):
        tok = tok_lists[e]
        n = len(tok)
        xg = np.zeros((C, D_MODEL), dtype=np.float32)
        xg[:n] = x_flat[tok]
        wt = np.zeros((C,), dtype=np.float32)
        we = np.where(idx[tok, 0] == e, w[tok, 0], w[tok, 1])
        wt[:n] = we

        xT_dev = np.ascontiguousarray(
            xg.reshape(C, KO1, P).transpose(2, 1, 0)
        ).reshape(P, KO1 * C).astype(BF16)
        w1_dev = np.ascontiguousarray(
            W1[e].reshape(KO1, P, NFT, P).transpose(1, 2, 0, 3)
        ).reshape(P, NFT * KO1 * P).astype(BF16)
        w2_dev = np.ascontiguousarray(
            W2[e].reshape(NFT, P, D_MODEL).transpose(1, 0, 2)
        ).reshape(P, NFT * D_MODEL).astype(BF16)
        b1_dev = np.ascontiguousarray(b1[e].reshape(NFT, P).T)
        cw_dev = np.ascontiguousarray(wt.reshape(C // P, P).T)

        in_maps.append({
            "xT": xT_dev,
            "w1": w1_dev,
            "w2": w2_dev,
            "b1": b1_dev.astype(np.float32),
            "cw": cw_dev.astype(np.float32),
        })

    from concourse.bass_utils import run_bass_kernel_spmd

    global LAST_IN_MAPS, LAST_C
    LAST_IN_MAPS = in_maps
    LAST_C = C

    trace = os.environ.get("MOE_KERNEL_TRACE", "0") == "1"
    res = run_bass_kernel_spmd(
        nc, in_maps, core_ids=list(range(N_CORES)),
        trace=trace, trace_cores=[0] if trace else None,
    )
    LAST_RESULTS = res

    # Unpack per-core outputs: y_dev [P, (C/P)*D] -> [C, D]
    Yall = np.empty((N_EXPERTS, C, D_MODEL), dtype=np.float32)
    for e in range(N_EXPERTS):
        y_dev = res.results[e]["y"]
        Yall[e] = (
            y_dev.reshape(P, C // P, D_MODEL)
            .transpose(1, 0, 2)
            .reshape(C, D_MODEL)
        )

    tok_all = np.arange(T)
    out_flat = (
        Yall[idx[:, 0], slot[tok_all, 0]] + Yall[idx[:, 1], slot[tok_all, 1]]
    )

    if np.any(b2):
        out_flat += w[:, 0:1] * b2[idx[:, 0]] + w[:, 1:2] * b2[idx[:, 1]]

    return out_flat.reshape(B, S, D_MODEL).astype(np.float32)
